# revision 7
# baseline (speedup 1.0000x reference)
"""Binarized 3x3 conv (BinarizeConv2dSDP) for one TRN2 chip (8 NeuronCores).

Reference computation:
    out = conv2d(sign(x), sign(M), stride=1, pad=1) * Alpha      (all fp32)
    x: (32, 256, 56, 56)   M: (256, 256, 3, 3)   Alpha: (256, 1, 1)

Strategy (per the data-parallel sharding hint):
  - Shard x over batch: 4 images per core; replicate M/Alpha on every core.
  - On-core: binarize x and M to fp8 (+/-1 exactly representable), run the
    conv as 9 shifted DoubleRow matmuls (contraction = 256 channels in one
    pass: 128 partitions x 2 pair-rows) accumulating in PSUM, scale by
    Alpha while evacuating PSUM, DMA out fp32.
  - Activations live in SBUF as zero-padded 58x58 images. Each matmul's
    moving AP is 2-level [8 rows x 56 cols] (row stride 58), so only the
    448 valid output columns of an 8-row strip are computed - no seam
    garbage (the ISA ifmap AP natively has a third dim for this).
  - Startup is latency-tuned: weights (ot=0) land first on the sync HWDGE
    queue, image-0 arrives as 9/26/23-row chunks (first chunks on the
    scalar HWDGE queue so both queues ramp in parallel), act-tile zero
    fills run on GpSimd off the DVE cast path, and a short warmup burst
    ramps the HAM clock gate so the first conv strip runs at 2.4 GHz.
"""

import time

import numpy as np

import concourse.bacc as bacc
import concourse.bass as bass
import concourse.tile as tile
from concourse import masks, mybir
from concourse.bass_utils import run_bass_kernel_spmd

F32 = mybir.dt.float32
BF16 = mybir.dt.bfloat16
FP8 = mybir.dt.float8e4

# ---- problem geometry (hardcoded; kernel.py must be self-contained) ----
N_CORES = 8
NB = 4          # images per core (32 / 8)
C = 256         # in channels  (2 halves of 128 partitions)
O = 256         # out channels (2 tiles of 128 partitions)
H = W = 56
K = 3
PW = H + 2      # padded row width  (58)
NPIX = PW * PW  # padded image size (3364)
PH = 3376       # padded image allocation (multiple of 16)
RS = 8          # output rows per strip
NSTRIP = H // RS        # 7
NVAL = RS * W           # 448 psum columns per strip (valid only)

# image-0 chunking (input-row ranges) for early conv start
U_R0, U_NR = 0, 9       # ultra chunk -> early tile, strip 0
B_R0, B_NR = 7, 26      # covers padded rows 8..33 (strips 1-3, + s4 top)
C_R0, C_NR = 33, 23     # covers padded rows 34..56 (strips 4-6)
EROWS = U_NR + 1        # early tile padded rows 0..9
EPH = 592               # 10*58=580 rounded up to a multiple of 16

N_WARM = 22             # warmup matmuls (HAM ramp; ends ~ when weights land)
OUT_SPLIT = 12          # out-DMAs before this gidx go via GpSimd SWDGE


def build_nc() -> bass.Bass:
    """Build the SPMD Bass program for one core's shard."""
    nc = bacc.Bacc("TRN2")

    x = nc.declare_dram_parameter("x", [NB, C, H, W], F32, isOutput=False)
    m = nc.declare_dram_parameter("m", [O, C, K, K], F32, isOutput=False)
    alpha = nc.declare_dram_parameter("alpha", [O], F32, isOutput=False)
    out = nc.declare_dram_parameter("out", [NB, O, H, W], F32, isOutput=True)

    with tile.TileContext(nc) as tc:
        with (
            tc.tile_pool(name="consts", bufs=1) as consts,
            tc.tile_pool(name="xsrc", bufs=6) as xsrc_pool,
            tc.tile_pool(name="xsrc0", bufs=6) as xsrc0_pool,
            tc.tile_pool(name="wsrc", bufs=2) as wsrc_pool,
            tc.tile_pool(name="wsgn", bufs=4) as wsgn_pool,
            tc.tile_pool(name="osb", bufs=8) as osb_pool,
            tc.tile_pool(name="ptr", bufs=2, space="PSUM") as ptr_pool,
            tc.tile_pool(name="pmm", bufs=4, space="PSUM") as pmm_pool,
        ):
            # alpha: two tiny scattered DMAs on the GpSimd SWDGE ring
            alpha_sb = consts.tile([128, 2], F32)
            for ot in range(2):
                nc.gpsimd.dma_start(
                    out=alpha_sb[:, ot : ot + 1],
                    in_=alpha.rearrange("(t o) -> t o", t=2)[ot].rearrange(
                        "(o u) -> o u", u=1
                    ),
                )

            # wz before any DVE dma dispatch: warmup matmuls gate on it
            wz = consts.tile([128, 256], BF16)
            nc.vector.memset(wz[:], 0)

            # ---- weight DMAs: one contiguous 1.18MB DMA per ot block ----
            # wsrc[o, c*9 + kk] = M[ot*128+o, c, kh, kw]
            def w_dma(ot):
                wsrc = wsrc_pool.tile([128, C * K * K], F32)
                nc.sync.dma_start(
                    out=wsrc[:],
                    in_=m[ot * 128 : (ot + 1) * 128].rearrange(
                        "o c kh kw -> o (c kh kw)"
                    ),
                )
                return (ot, wsrc)

            def x_chunk_dma(eng, n, half, r0, nr):
                xs = xsrc0_pool.tile([128, B_NR * W], F32)
                eng.dma_start(
                    out=xs[:, : nr * W],
                    in_=x[n, half * 128 : (half + 1) * 128, r0 : r0 + nr, :].rearrange(
                        "c h w -> c (h w)"
                    ),
                )
                return xs

            def x_img_dma(eng, n, half):
                xs = xsrc_pool.tile([128, H * W], F32)
                eng.dma_start(
                    out=xs[:],
                    in_=x[n, half * 128 : (half + 1) * 128].rearrange(
                        "c h w -> c (h w)"
                    ),
                )
                return (n, half, xs)

            # sync queue: w(ot=0) first (longest dep chain), then w(ot=1),
            # image-0 bottom chunk, images 2-3.
            wprep = [w_dma(0)]
            # scalar queue ramps in parallel: image-0 top chunks + image 1
            xu = [x_chunk_dma(nc.scalar, 0, h2, U_R0, U_NR) for h2 in range(2)]
            xb = [x_chunk_dma(nc.scalar, 0, h2, B_R0, B_NR) for h2 in range(2)]
            wprep.append(w_dma(1))
            xc = [x_chunk_dma(nc.sync, 0, h2, C_R0, C_NR) for h2 in range(2)]
            xtiles = [x_img_dma(nc.scalar, 1, h2) for h2 in range(2)]
            for n in range(2, NB):
                for h2 in range(2):
                    xtiles.append(x_img_dma(nc.sync, n, h2))

            # ---- PE warm-up: ramps the HAM clock gate (K=4/8 -> 8/8)
            # while the weight DMA + sign land ----
            pwarm = pmm_pool.tile([128, NVAL], F32, tag="pm")
            for _ in range(N_WARM):
                nc.tensor.matmul(
                    pwarm[:, :256], wz[:, :128], wz[:], start=True, stop=True
                )

            identity = consts.tile([128, 128], BF16)
            masks.make_identity(nc, identity[:])

            # activation tiles: zero fill on GpSimd (keeps DVE free for the
            # weight-cast stream). Image 0 + early tile + image 1 now;
            # images 2-3 deferred into the conv loop.
            act = consts.tile([128, 2 * NB, PH], FP8)
            acte = consts.tile([128, 2, EPH], FP8)
            nc.gpsimd.memset(
                acte.rearrange("p a b -> p (a b)").bitcast(mybir.dt.uint32), 0
            )

            def act_memset(n):
                nc.gpsimd.memset(
                    act[:, 2 * n : 2 * n + 2, :]
                    .rearrange("p a b -> p (a b)")
                    .bitcast(mybir.dt.uint32),
                    0,
                )

            act_memset(0)
            act_memset(1)

            # ---- weight prep: sign (ACT), 9 PE transposes, 9 DVE casts ----
            # wbuf[c2, half, kk*256 + ot*128 + o] = sign(M[ot*128+o, half*128+c2, kh, kw])
            wbuf = consts.tile([128, 2, K * K * O], FP8)

            def w_unit(ot, wsrc, half):
                """Tap-major chunked sign (transposes start right after the
                first chunk), 8 transposes staged into one PSUM bank and
                evacuated with a single wide cast (+1 small for tap 8)."""
                wsgn = wsgn_pool.tile([128, 128 * K * K], BF16)
                wsgn_ck = wsgn.rearrange("o (c k) -> o c k", k=K * K)
                wsrc_ck = wsrc[:, half * 1152 : (half + 1) * 1152].rearrange(
                    "o (c k) -> o c k", k=K * K
                )
                for c0 in range(0, K * K, 3):
                    nc.scalar.sign(
                        wsgn_ck[:, :, c0 : c0 + 3], wsrc_ck[:, :, c0 : c0 + 3]
                    )
                wide = ptr_pool.tile([128, 8 * 128], BF16)
                for kk in range(8):
                    nc.tensor.transpose(
                        wide[:, kk * 128 : (kk + 1) * 128],
                        wsgn_ck[:, :, kk],
                        identity[:],
                    )
                tp8 = ptr_pool.tile([128, 128], BF16)
                nc.tensor.transpose(tp8[:], wsgn_ck[:, :, 8], identity[:])
                dst8 = wbuf[:, half, ot * 128 : ot * 128 + 8 * O].rearrange(
                    "p (k o) -> p k o", o=O
                )[:, :, :128]
                nc.vector.tensor_copy(
                    dst8, wide.rearrange("p (k o) -> p k o", o=128)
                )
                nc.vector.tensor_copy(
                    wbuf[:, half, 8 * O + ot * 128 : 8 * O + ot * 128 + 128], tp8[:]
                )

            w_unit(0, wprep[0][1], 0)
            w_unit(0, wprep[0][1], 1)

            # ---- image-0 signs: ultra -> early tile, B/C -> main act ----
            for h2 in range(2):
                dst = acte[:, h2, : EROWS * PW].rearrange("p (h w) -> p h w", w=PW)[
                    :, 1 : 1 + U_NR, 1 : W + 1
                ]
                nc.scalar.sign(
                    dst, xu[h2][:, : U_NR * W].rearrange("p (h w) -> p h w", w=W)
                )
            # B in two row chunks so strip 1 unblocks before all of B signs
            for r0, r1 in ((0, 13), (13, B_NR)):
                for h2 in range(2):
                    dst = act[:, h2, :NPIX].rearrange("p (h w) -> p h w", w=PW)[
                        :, 1 + B_R0 + r0 : 1 + B_R0 + r1, 1 : W + 1
                    ]
                    nc.scalar.sign(
                        dst,
                        xb[h2][:, r0 * W : r1 * W].rearrange(
                            "p (h w) -> p h w", w=W
                        ),
                    )
            for h2 in range(2):
                dst = act[:, h2, :NPIX].rearrange("p (h w) -> p h w", w=PW)[
                    :, 1 + C_R0 : 1 + C_R0 + C_NR, 1 : W + 1
                ]
                nc.scalar.sign(
                    dst, xc[h2][:, : C_NR * W].rearrange("p (h w) -> p h w", w=W)
                )

            def sign_img(n, half, xs):
                dst = act[:, 2 * n + half, :NPIX].rearrange(
                    "p (h w) -> p h w", w=PW
                )[:, 1 : H + 1, 1 : W + 1]
                nc.scalar.sign(dst, xs.rearrange("p (h w) -> p h w", w=W))

            # deferred work hooks: (gidx -> thunk) slipped into the conv loop
            wunits = [
                lambda: w_unit(1, wprep[1][1], 0),
                lambda: w_unit(1, wprep[1][1], 1),
            ]
            imgsigns = [lambda t=t: sign_img(*t) for t in xtiles]
            hooks = {
                3: wunits[0],
                5: wunits[1],
                6: imgsigns[0],
                7: imgsigns[1],
                8: lambda: act_memset(2),
                10: imgsigns[2],
                11: imgsigns[3],
                12: lambda: act_memset(3),
                14: imgsigns[4],
                15: imgsigns[5],
            }

            # ---- main conv loop: image-major, ot inner ----
            def conv_strip(n, ot, s, gidx, r0=0, nr=RS):
                pm = pmm_pool.tile([128, nr * W], F32)
                for kk in range(K * K):
                    kh, kw = divmod(kk, K)
                    base = (RS * s + r0 + kh) * PW + kw
                    lhsT2 = wbuf[:, :, kk * O + ot * 128 : kk * O + ot * 128 + 128]
                    src = acte if (n == 0 and s == 0) else act[:, 2 * n : 2 * n + 2]
                    # 2-level moving AP: nr rows x 56 valid cols
                    rhs4 = src[:, :, base : base + nr * PW].rearrange(
                        "p a (r w) -> p a r w", w=PW
                    )[:, :, :, :W]
                    nc.tensor.matmul(
                        pm[:],
                        lhsT2,
                        rhs4,
                        start=(kk == 0),
                        stop=(kk == K * K - 1),
                        perf_mode=mybir.MatmulPerfMode.DoubleRow,
                    )
                # evacuate, scaled by per-channel alpha
                osb = osb_pool.tile([128, nr * W], F32)
                nc.vector.tensor_scalar_mul(osb[:], pm[:], alpha_sb[:, ot : ot + 1])
                # early outs via GpSimd SWDGE (sync HWDGE is busy with
                # inputs); late outs via the then-idle sync ring
                eng = nc.gpsimd if gidx < OUT_SPLIT else nc.sync
                eng.dma_start(
                    out=out[
                        n,
                        ot * 128 : (ot + 1) * 128,
                        RS * s + r0 : RS * s + r0 + nr,
                        :,
                    ].rearrange("o h w -> o (h w)"),
                    in_=osb[:],
                )

            gidx = 0
            for n in range(NB):
                for ot in range(2):
                    for s in range(NSTRIP):
                        if gidx == 2 * NB * NSTRIP - 1:
                            # split the final strip: the first half's
                            # evac+DMA overlaps the second half's matmuls
                            conv_strip(n, ot, s, gidx, 0, RS // 2)
                            conv_strip(n, ot, s, gidx, RS // 2, RS // 2)
                        else:
                            conv_strip(n, ot, s, gidx)
                        if gidx in hooks:
                            hooks[gidx]()
                        gidx += 1
    nc.finalize()
    return nc


_NC_CACHE: dict[bool, bass.Bass] = {}


def get_nc(paired: bool = True) -> bass.Bass:
    if paired not in _NC_CACHE:
        _NC_CACHE[paired] = build_nc()
    return _NC_CACHE[paired]


def kernel(x: np.ndarray, M: np.ndarray, Alpha: np.ndarray) -> np.ndarray:
    """Full (unsharded) inputs in, full output out. Runs on 8 NeuronCores."""
    assert x.shape == (N_CORES * NB, C, H, W), x.shape
    nc = get_nc()
    x = np.ascontiguousarray(x, dtype=np.float32)
    M = np.ascontiguousarray(M, dtype=np.float32)
    a = np.ascontiguousarray(Alpha, dtype=np.float32).reshape(O)
    in_maps = [
        {"x": x[i * NB : (i + 1) * NB], "m": M, "alpha": a} for i in range(N_CORES)
    ]
    last_err = None
    for attempt in range(3):
        try:
            res = run_bass_kernel_spmd(nc, in_maps, list(range(N_CORES)))
            break
        except Exception as e:  # transient NRT/axon faults recover on retry
            last_err = e
            time.sleep(10 * (attempt + 1))
    else:
        raise last_err
    return np.concatenate([res.results[i]["out"] for i in range(N_CORES)], axis=0)


# revision 14
# speedup vs baseline: 1.0142x; 1.0142x over previous
"""Binarized 3x3 conv (BinarizeConv2dSDP) for one TRN2 chip (8 NeuronCores).

Reference computation:
    out = conv2d(sign(x), sign(M), stride=1, pad=1) * Alpha      (all fp32)
    x: (32, 256, 56, 56)   M: (256, 256, 3, 3)   Alpha: (256, 1, 1)

Strategy (per the data-parallel sharding hint):
  - Shard x over batch: 4 images per core; replicate M/Alpha on every core.
  - On-core: binarize x and M to fp8 (+/-1 exactly representable), run the
    conv as 9 shifted DoubleRow matmuls (contraction = 256 channels in one
    pass: 128 partitions x 2 pair-rows) accumulating in PSUM, scale by
    Alpha while evacuating PSUM, DMA out fp32.
  - Activations live in SBUF as zero-padded 58x58 images. Each matmul's
    moving AP is 2-level [8 rows x 56 cols] (row stride 58), so only the
    448 valid output columns of an 8-row strip are computed - no seam
    garbage (the ISA ifmap AP natively has a third dim for this).
  - Startup is latency-tuned: weights (ot=0) land first on the sync HWDGE
    queue, image-0 arrives as 9/26/23-row chunks (first chunks on the
    scalar HWDGE queue so both queues ramp in parallel), act-tile zero
    fills run on GpSimd off the DVE cast path, and a short warmup burst
    ramps the HAM clock gate so the first conv strip runs at 2.4 GHz.
"""

import time

import numpy as np

import concourse.bacc as bacc
import concourse.bass as bass
import concourse.tile as tile
from concourse import masks, mybir
from concourse.bass_utils import run_bass_kernel_spmd

F32 = mybir.dt.float32
BF16 = mybir.dt.bfloat16
FP8 = mybir.dt.float8e4

# ---- problem geometry (hardcoded; kernel.py must be self-contained) ----
N_CORES = 8
NB = 4          # images per core (32 / 8)
C = 256         # in channels  (2 halves of 128 partitions)
O = 256         # out channels (2 tiles of 128 partitions)
H = W = 56
K = 3
PW = H + 2      # padded row width  (58)
NPIX = PW * PW  # padded image size (3364)
PH = 3376       # padded image allocation (multiple of 16)
RS = 8          # output rows per strip
NSTRIP = H // RS        # 7
NVAL = RS * W           # 448 psum columns per strip (valid only)

# image-0 chunking (input-row ranges) for early conv start
U_R0, U_NR = 0, 9       # ultra chunk -> early tile, strip 0
B_R0, B_NR = 7, 26      # covers padded rows 8..33 (strips 1-3, + s4 top)
C_R0, C_NR = 33, 23     # covers padded rows 34..56 (strips 4-6)
EROWS = U_NR + 1        # early tile padded rows 0..9
EPH = 592               # 10*58=580 rounded up to a multiple of 16

N_WARM = 26             # warmup matmuls (HAM ramp; ends ~ when weights land)
OUT_SPLIT = 12          # out-DMAs before this gidx go via GpSimd SWDGE


def build_nc() -> bass.Bass:
    """Build the SPMD Bass program for one core's shard."""
    nc = bacc.Bacc("TRN2")

    x = nc.declare_dram_parameter("x", [NB, C, H, W], F32, isOutput=False)
    m = nc.declare_dram_parameter("m", [O, C, K, K], F32, isOutput=False)
    alpha = nc.declare_dram_parameter("alpha", [O], F32, isOutput=False)
    out = nc.declare_dram_parameter("out", [NB, O, H, W], F32, isOutput=True)

    with tile.TileContext(nc) as tc:
        with (
            tc.tile_pool(name="consts", bufs=1) as consts,
            tc.tile_pool(name="xsrc", bufs=6) as xsrc_pool,
            tc.tile_pool(name="xsrc0", bufs=6) as xsrc0_pool,
            tc.tile_pool(name="wsrc", bufs=4) as wsrc_pool,
            tc.tile_pool(name="wsgn", bufs=4) as wsgn_pool,
            tc.tile_pool(name="osb", bufs=8) as osb_pool,
            tc.tile_pool(name="ptr", bufs=2, space="PSUM") as ptr_pool,
            tc.tile_pool(name="pmm", bufs=4, space="PSUM") as pmm_pool,
        ):
            # alpha: two tiny scattered DMAs on the GpSimd SWDGE ring
            alpha_sb = consts.tile([128, 2], F32)
            for ot in range(2):
                nc.gpsimd.dma_start(
                    out=alpha_sb[:, ot : ot + 1],
                    in_=alpha.rearrange("(t o) -> t o", t=2)[ot].rearrange(
                        "(o u) -> o u", u=1
                    ),
                )

            # wz before any DVE dma dispatch: warmup matmuls gate on it
            wz = consts.tile([128, 256], BF16)
            nc.vector.memset(wz[:], 0)

            # ---- weight DMAs ----
            # wsrc[o, c*9 + kk] = M[ot*128+o, half*128+c, kh, kw]
            # ot=0 halves land in parallel on the sync + scalar queues (the
            # startup critical path); ot=1 is one sync DMA, not critical.
            def w_dma_half(eng, ot, half):
                wsrc = wsrc_pool.tile([128, 128 * K * K], F32)
                eng.dma_start(
                    out=wsrc[:],
                    in_=m[
                        ot * 128 : (ot + 1) * 128, half * 128 : (half + 1) * 128
                    ].rearrange("o c kh kw -> o (c kh kw)"),
                )
                return wsrc

            def x_chunk_dma(eng, n, half, r0, nr):
                xs = xsrc0_pool.tile([128, B_NR * W], F32)
                eng.dma_start(
                    out=xs[:, : nr * W],
                    in_=x[n, half * 128 : (half + 1) * 128, r0 : r0 + nr, :].rearrange(
                        "c h w -> c (h w)"
                    ),
                )
                return xs

            def x_img_dma(eng, n, half):
                xs = xsrc_pool.tile([128, H * W], F32)
                eng.dma_start(
                    out=xs[:],
                    in_=x[n, half * 128 : (half + 1) * 128].rearrange(
                        "c h w -> c (h w)"
                    ),
                )
                return (n, half, xs)

            # sync queue: w(ot=0) h0 first (longest dep chain), then w(ot=1),
            # image-0 bottom chunk, images 2-3.
            w0 = [w_dma_half(nc.sync, 0, 0)]
            # scalar queue ramps in parallel: w(ot=0) h1, then image-0 top
            # chunks + image 1
            w0.append(w_dma_half(nc.scalar, 0, 1))
            xu = [x_chunk_dma(nc.scalar, 0, h2, U_R0, U_NR) for h2 in range(2)]
            xb = [x_chunk_dma(nc.scalar, 0, h2, B_R0, B_NR) for h2 in range(2)]
            w1 = [w_dma_half(nc.sync, 1, 0), w_dma_half(nc.sync, 1, 1)]
            xc = [x_chunk_dma(nc.sync, 0, h2, C_R0, C_NR) for h2 in range(2)]
            xtiles = [x_img_dma(nc.scalar, 1, h2) for h2 in range(2)]
            for n in range(2, NB):
                for h2 in range(2):
                    xtiles.append(x_img_dma(nc.sync, n, h2))

            # ---- PE warm-up: ramps the HAM clock gate (K=4/8 -> 8/8)
            # while the weight DMA + sign land ----
            pwarm = pmm_pool.tile([128, NVAL], F32, tag="pm")
            for _ in range(N_WARM):
                nc.tensor.matmul(
                    pwarm[:, :256], wz[:, :128], wz[:], start=True, stop=True
                )

            identity = consts.tile([128, 128], BF16)
            masks.make_identity(nc, identity[:])

            # activation tiles: zero fill on GpSimd (keeps DVE free for the
            # weight-cast stream). Image 0 + early tile + image 1 now;
            # images 2-3 deferred into the conv loop.
            act = consts.tile([128, 2 * NB, PH], FP8)
            acte = consts.tile([128, 2, EPH], FP8)
            nc.gpsimd.memset(
                acte.rearrange("p a b -> p (a b)").bitcast(mybir.dt.uint32), 0
            )

            def act_memset(n):
                nc.gpsimd.memset(
                    act[:, 2 * n : 2 * n + 2, :]
                    .rearrange("p a b -> p (a b)")
                    .bitcast(mybir.dt.uint32),
                    0,
                )

            act_memset(0)
            act_memset(1)

            # ---- weight prep: sign (ACT), 9 PE transposes, 9 DVE casts ----
            # wbuf[c2, half, kk*256 + ot*128 + o] = sign(M[ot*128+o, half*128+c2, kh, kw])
            wbuf = consts.tile([128, 2, K * K * O], FP8)

            def w_unit(ot, wsrc, half):
                """Contiguous sign; tap 0 transposed + cast first (the conv
                consumes taps in order), taps 1-8 staged into one PSUM bank
                and evacuated with a single wide cast."""
                wsgn = wsgn_pool.tile([128, 128 * K * K], BF16)
                nc.scalar.sign(wsgn[:], wsrc[:])
                wsgn_ck = wsgn.rearrange("o (c k) -> o c k", k=K * K)
                tp0 = ptr_pool.tile([128, 128], BF16)
                nc.tensor.transpose(tp0[:], wsgn_ck[:, :, 0], identity[:])
                nc.vector.tensor_copy(
                    wbuf[:, half, ot * 128 : ot * 128 + 128], tp0[:]
                )
                wide = ptr_pool.tile([128, 8 * 128], BF16)
                for kk in range(1, K * K):
                    nc.tensor.transpose(
                        wide[:, (kk - 1) * 128 : kk * 128],
                        wsgn_ck[:, :, kk],
                        identity[:],
                    )
                dst8 = wbuf[:, half, :].rearrange("p (k o) -> p k o", o=O)[
                    :, 1:, ot * 128 : ot * 128 + 128
                ]
                nc.vector.tensor_copy(
                    dst8, wide.rearrange("p (k o) -> p k o", o=128)
                )

            w_unit(0, w0[0], 0)
            w_unit(0, w0[1], 1)

            # ---- image-0 signs: ultra -> early tile, B/C -> main act ----
            for h2 in range(2):
                dst = acte[:, h2, : EROWS * PW].rearrange("p (h w) -> p h w", w=PW)[
                    :, 1 : 1 + U_NR, 1 : W + 1
                ]
                nc.scalar.sign(
                    dst, xu[h2][:, : U_NR * W].rearrange("p (h w) -> p h w", w=W)
                )
            # B in two row chunks so strip 1 unblocks before all of B signs
            for r0, r1 in ((0, 13), (13, B_NR)):
                for h2 in range(2):
                    dst = act[:, h2, :NPIX].rearrange("p (h w) -> p h w", w=PW)[
                        :, 1 + B_R0 + r0 : 1 + B_R0 + r1, 1 : W + 1
                    ]
                    nc.scalar.sign(
                        dst,
                        xb[h2][:, r0 * W : r1 * W].rearrange(
                            "p (h w) -> p h w", w=W
                        ),
                    )
            for h2 in range(2):
                dst = act[:, h2, :NPIX].rearrange("p (h w) -> p h w", w=PW)[
                    :, 1 + C_R0 : 1 + C_R0 + C_NR, 1 : W + 1
                ]
                nc.scalar.sign(
                    dst, xc[h2][:, : C_NR * W].rearrange("p (h w) -> p h w", w=W)
                )

            def sign_img(n, half, xs):
                dst = act[:, 2 * n + half, :NPIX].rearrange(
                    "p (h w) -> p h w", w=PW
                )[:, 1 : H + 1, 1 : W + 1]
                nc.scalar.sign(dst, xs.rearrange("p (h w) -> p h w", w=W))

            # deferred work hooks: (gidx -> thunk) slipped into the conv loop
            wunits = [
                lambda: w_unit(1, w1[0], 0),
                lambda: w_unit(1, w1[1], 1),
            ]
            imgsigns = [lambda t=t: sign_img(*t) for t in xtiles]
            hooks = {
                3: wunits[0],
                5: wunits[1],
                6: imgsigns[0],
                7: imgsigns[1],
                8: lambda: act_memset(2),
                10: imgsigns[2],
                11: imgsigns[3],
                12: lambda: act_memset(3),
                14: imgsigns[4],
                15: imgsigns[5],
            }

            # ---- main conv loop: image-major, ot inner ----
            def conv_strip(n, ot, s, gidx, r0=0, nr=RS):
                pm = pmm_pool.tile([128, nr * W], F32)
                for kk in range(K * K):
                    kh, kw = divmod(kk, K)
                    base = (RS * s + r0 + kh) * PW + kw
                    lhsT2 = wbuf[:, :, kk * O + ot * 128 : kk * O + ot * 128 + 128]
                    src = acte if (n == 0 and s == 0) else act[:, 2 * n : 2 * n + 2]
                    # 2-level moving AP: nr rows x 56 valid cols
                    rhs4 = src[:, :, base : base + nr * PW].rearrange(
                        "p a (r w) -> p a r w", w=PW
                    )[:, :, :, :W]
                    nc.tensor.matmul(
                        pm[:],
                        lhsT2,
                        rhs4,
                        start=(kk == 0),
                        stop=(kk == K * K - 1),
                        perf_mode=mybir.MatmulPerfMode.DoubleRow,
                    )
                # evacuate, scaled by per-channel alpha
                osb = osb_pool.tile([128, nr * W], F32)
                nc.vector.tensor_scalar_mul(osb[:], pm[:], alpha_sb[:, ot : ot + 1])
                # early outs via GpSimd SWDGE (sync HWDGE is busy with
                # inputs); late outs via the then-idle sync ring
                eng = nc.gpsimd if gidx < OUT_SPLIT else nc.sync
                eng.dma_start(
                    out=out[
                        n,
                        ot * 128 : (ot + 1) * 128,
                        RS * s + r0 : RS * s + r0 + nr,
                        :,
                    ].rearrange("o h w -> o (h w)"),
                    in_=osb[:],
                )

            gidx = 0
            for n in range(NB):
                for ot in range(2):
                    for s in range(NSTRIP):
                        if gidx == 2 * NB * NSTRIP - 1:
                            # split the final strip: the first half's
                            # evac+DMA overlaps the second half's matmuls
                            conv_strip(n, ot, s, gidx, 0, RS // 2)
                            conv_strip(n, ot, s, gidx, RS // 2, RS // 2)
                        else:
                            conv_strip(n, ot, s, gidx)
                        if gidx in hooks:
                            hooks[gidx]()
                        gidx += 1
    nc.finalize()
    return nc


_NC_CACHE: dict[bool, bass.Bass] = {}


def get_nc(paired: bool = True) -> bass.Bass:
    if paired not in _NC_CACHE:
        _NC_CACHE[paired] = build_nc()
    return _NC_CACHE[paired]


def kernel(x: np.ndarray, M: np.ndarray, Alpha: np.ndarray) -> np.ndarray:
    """Full (unsharded) inputs in, full output out. Runs on 8 NeuronCores."""
    assert x.shape == (N_CORES * NB, C, H, W), x.shape
    nc = get_nc()
    x = np.ascontiguousarray(x, dtype=np.float32)
    M = np.ascontiguousarray(M, dtype=np.float32)
    a = np.ascontiguousarray(Alpha, dtype=np.float32).reshape(O)
    in_maps = [
        {"x": x[i * NB : (i + 1) * NB], "m": M, "alpha": a} for i in range(N_CORES)
    ]
    last_err = None
    for attempt in range(3):
        try:
            res = run_bass_kernel_spmd(nc, in_maps, list(range(N_CORES)))
            break
        except Exception as e:  # transient NRT/axon faults recover on retry
            last_err = e
            time.sleep(10 * (attempt + 1))
    else:
        raise last_err
    return np.concatenate([res.results[i]["out"] for i in range(N_CORES)], axis=0)


# revision 18
# speedup vs baseline: 1.0526x; 1.0379x over previous
"""Binarized 3x3 conv (BinarizeConv2dSDP) for one TRN2 chip (8 NeuronCores).

Reference computation:
    out = conv2d(sign(x), sign(M), stride=1, pad=1) * Alpha      (all fp32)
    x: (32, 256, 56, 56)   M: (256, 256, 3, 3)   Alpha: (256, 1, 1)

Strategy (per the data-parallel sharding hint):
  - Shard x over batch: 4 images per core; replicate M/Alpha on every core.
  - On-core: binarize x and M to fp8 (+/-1 exactly representable), run the
    conv as 9 shifted DoubleRow matmuls (contraction = 256 channels in one
    pass: 128 partitions x 2 pair-rows) accumulating in PSUM, scale by
    Alpha while evacuating PSUM, DMA out fp32.
  - Activations live in SBUF as zero-padded 58x58 images. Each matmul's
    moving AP is 2-level [8 rows x 56 cols] (row stride 58), so only the
    448 valid output columns of an 8-row strip are computed - no seam
    garbage (the ISA ifmap AP natively has a third dim for this).
  - Startup is latency-tuned: weights (ot=0) land first on the sync HWDGE
    queue, image-0 arrives as 9/26/23-row chunks (first chunks on the
    scalar HWDGE queue so both queues ramp in parallel), act-tile zero
    fills run on GpSimd off the DVE cast path, and a short warmup burst
    ramps the HAM clock gate so the first conv strip runs at 2.4 GHz.
"""

import time

import numpy as np

import concourse.bacc as bacc
import concourse.bass as bass
import concourse.tile as tile
from concourse import masks, mybir
from concourse.bass_utils import run_bass_kernel_spmd

F32 = mybir.dt.float32
BF16 = mybir.dt.bfloat16
FP8 = mybir.dt.float8e4

# ---- problem geometry (hardcoded; kernel.py must be self-contained) ----
N_CORES = 8
NB = 4          # images per core (32 / 8)
C = 256         # in channels  (2 halves of 128 partitions)
O = 256         # out channels (2 tiles of 128 partitions)
H = W = 56
K = 3
PW = H + 2      # padded row width  (58)
NPIX = PW * PW  # padded image size (3364)
PH = 3376       # padded image allocation (multiple of 16)
RS = 8          # output rows per strip
NSTRIP = H // RS        # 7
NVAL = RS * W           # 448 psum columns per strip (valid only)

# image-0 chunking (input-row ranges) for early conv start
U_R0, U_NR = 0, 9       # ultra chunk -> early tile, strip 0
B_R0, B_NR = 7, 26      # covers padded rows 8..33 (strips 1-3, + s4 top)
C_R0, C_NR = 33, 23     # covers padded rows 34..56 (strips 4-6)
EROWS = U_NR + 1        # early tile padded rows 0..9
EPH = 592               # 10*58=580 rounded up to a multiple of 16

N_WARM = 22             # warmup matmuls (HAM ramp; ends ~ when weights land)
OUT_SPLIT = 12          # out-DMAs before this gidx go via GpSimd SWDGE


def build_nc() -> bass.Bass:
    """Build the SPMD Bass program for one core's shard."""
    nc = bacc.Bacc("TRN2")

    x = nc.declare_dram_parameter("x", [NB, C, H, W], F32, isOutput=False)
    m = nc.declare_dram_parameter("m", [O, C, K, K], F32, isOutput=False)
    alpha = nc.declare_dram_parameter("alpha", [O], F32, isOutput=False)
    out = nc.declare_dram_parameter("out", [NB, O, H, W], F32, isOutput=True)

    with tile.TileContext(nc) as tc:
        with (
            tc.tile_pool(name="consts", bufs=1) as consts,
            tc.tile_pool(name="xsrc", bufs=6) as xsrc_pool,
            tc.tile_pool(name="xsrc0", bufs=6) as xsrc0_pool,
            tc.tile_pool(name="wsrc", bufs=4) as wsrc_pool,
            tc.tile_pool(name="wsgn", bufs=4) as wsgn_pool,
            tc.tile_pool(name="osb", bufs=8) as osb_pool,
            tc.tile_pool(name="ptr", bufs=2, space="PSUM") as ptr_pool,
            tc.tile_pool(name="pmm", bufs=4, space="PSUM") as pmm_pool,
        ):
            # alpha: two tiny scattered DMAs on the GpSimd SWDGE ring
            alpha_sb = consts.tile([128, 2], F32)
            for ot in range(2):
                nc.gpsimd.dma_start(
                    out=alpha_sb[:, ot : ot + 1],
                    in_=alpha.rearrange("(t o) -> t o", t=2)[ot].rearrange(
                        "(o u) -> o u", u=1
                    ),
                )

            # wz before any DVE dma dispatch: warmup matmuls gate on it
            wz = consts.tile([128, 256], BF16)
            nc.vector.memset(wz[:], 0)

            # ---- weight DMAs ----
            # wsrc[o, c*9 + kk] = M[ot*128+o, half*128+c, kh, kw]
            # ot=0 halves land in parallel on the sync + scalar queues (the
            # startup critical path); ot=1 is one sync DMA, not critical.
            def w_dma_half(eng, ot, half):
                wsrc = wsrc_pool.tile([128, 128 * K * K], F32)
                eng.dma_start(
                    out=wsrc[:],
                    in_=m[
                        ot * 128 : (ot + 1) * 128, half * 128 : (half + 1) * 128
                    ].rearrange("o c kh kw -> o (c kh kw)"),
                )
                return wsrc

            def x_chunk_dma(eng, n, half, r0, nr):
                xs = xsrc0_pool.tile([128, B_NR * W], F32)
                eng.dma_start(
                    out=xs[:, : nr * W],
                    in_=x[n, half * 128 : (half + 1) * 128, r0 : r0 + nr, :].rearrange(
                        "c h w -> c (h w)"
                    ),
                )
                return xs

            def x_img_dma(eng, n, half):
                xs = xsrc_pool.tile([128, H * W], F32)
                eng.dma_start(
                    out=xs[:],
                    in_=x[n, half * 128 : (half + 1) * 128].rearrange(
                        "c h w -> c (h w)"
                    ),
                )
                return (n, half, xs)

            # sync queue (fastest ramp): w(ot=0) halves back-to-back so the
            # h0 sign starts ~3us before h1 lands, then w(ot=1) + image-0
            # bottom chunk. Images 1-3 are dispatched from conv-loop hooks:
            # dispatching them here would exhaust HWDGE ring credit and the
            # blocked dispatch would sit ahead of the critical w signs.
            w0 = [w_dma_half(nc.sync, 0, 0), w_dma_half(nc.sync, 0, 1)]
            # scalar queue ramps in parallel with image-0 top chunks
            xu = [x_chunk_dma(nc.scalar, 0, h2, U_R0, U_NR) for h2 in range(2)]
            xb = [x_chunk_dma(nc.scalar, 0, h2, B_R0, B_NR) for h2 in range(2)]
            w1 = [w_dma_half(nc.sync, 1, 0), w_dma_half(nc.sync, 1, 1)]
            xc = [x_chunk_dma(nc.sync, 0, h2, C_R0, C_NR) for h2 in range(2)]
            xtiles = []

            # ---- PE warm-up: ramps the HAM clock gate (K=4/8 -> 8/8)
            # while the weight DMA + sign land ----
            pwarm = pmm_pool.tile([128, NVAL], F32, tag="pm")
            for _ in range(N_WARM):
                nc.tensor.matmul(
                    pwarm[:, :256], wz[:, :128], wz[:], start=True, stop=True
                )

            identity = consts.tile([128, 128], BF16)
            masks.make_identity(nc, identity[:])

            # activation tiles: zero fill on GpSimd (keeps DVE free for the
            # weight-cast stream). Image 0 + early tile + image 1 now;
            # images 2-3 deferred into the conv loop.
            act = consts.tile([128, 2 * NB, PH], FP8)
            acte = consts.tile([128, 2, EPH], FP8)
            nc.gpsimd.memset(
                acte.rearrange("p a b -> p (a b)").bitcast(mybir.dt.uint32), 0
            )

            def act_memset(n):
                nc.gpsimd.memset(
                    act[:, 2 * n : 2 * n + 2, :]
                    .rearrange("p a b -> p (a b)")
                    .bitcast(mybir.dt.uint32),
                    0,
                )

            act_memset(0)
            act_memset(1)

            # ---- weight prep: sign (ACT), 9 PE transposes, 9 DVE casts ----
            # wbuf[c2, half, kk*256 + ot*128 + o] = sign(M[ot*128+o, half*128+c2, kh, kw])
            wbuf = consts.tile([128, 2, K * K * O], FP8)

            def w_unit(ot, wsrc, half):
                """Contiguous sign; tap 0 transposed + cast first (the conv
                consumes taps in order), taps 1-8 staged into one PSUM bank
                and evacuated with a single wide cast."""
                wsgn = wsgn_pool.tile([128, 128 * K * K], BF16)
                nc.scalar.sign(wsgn[:], wsrc[:])
                wsgn_ck = wsgn.rearrange("o (c k) -> o c k", k=K * K)
                tp0 = ptr_pool.tile([128, 128], BF16)
                nc.tensor.transpose(tp0[:], wsgn_ck[:, :, 0], identity[:])
                nc.vector.tensor_copy(
                    wbuf[:, half, ot * 128 : ot * 128 + 128], tp0[:]
                )
                wide = ptr_pool.tile([128, 8 * 128], BF16)
                for kk in range(1, K * K):
                    nc.tensor.transpose(
                        wide[:, (kk - 1) * 128 : kk * 128],
                        wsgn_ck[:, :, kk],
                        identity[:],
                    )
                dst8 = wbuf[:, half, :].rearrange("p (k o) -> p k o", o=O)[
                    :, 1:, ot * 128 : ot * 128 + 128
                ]
                nc.vector.tensor_copy(
                    dst8, wide.rearrange("p (k o) -> p k o", o=128)
                )

            w_unit(0, w0[0], 0)
            w_unit(0, w0[1], 1)

            # ---- image-0 signs: ultra -> early tile, B/C -> main act ----
            for h2 in range(2):
                dst = acte[:, h2, : EROWS * PW].rearrange("p (h w) -> p h w", w=PW)[
                    :, 1 : 1 + U_NR, 1 : W + 1
                ]
                nc.scalar.sign(
                    dst, xu[h2][:, : U_NR * W].rearrange("p (h w) -> p h w", w=W)
                )
            # B in two row chunks so strip 1 unblocks before all of B signs
            for r0, r1 in ((0, 13), (13, B_NR)):
                for h2 in range(2):
                    dst = act[:, h2, :NPIX].rearrange("p (h w) -> p h w", w=PW)[
                        :, 1 + B_R0 + r0 : 1 + B_R0 + r1, 1 : W + 1
                    ]
                    nc.scalar.sign(
                        dst,
                        xb[h2][:, r0 * W : r1 * W].rearrange(
                            "p (h w) -> p h w", w=W
                        ),
                    )
            for h2 in range(2):
                dst = act[:, h2, :NPIX].rearrange("p (h w) -> p h w", w=PW)[
                    :, 1 + C_R0 : 1 + C_R0 + C_NR, 1 : W + 1
                ]
                nc.scalar.sign(
                    dst, xc[h2][:, : C_NR * W].rearrange("p (h w) -> p h w", w=W)
                )

            def sign_img(n, half, xs):
                dst = act[:, 2 * n + half, :NPIX].rearrange(
                    "p (h w) -> p h w", w=PW
                )[:, 1 : H + 1, 1 : W + 1]
                nc.scalar.sign(dst, xs.rearrange("p (h w) -> p h w", w=W))

            # deferred work hooks: (gidx -> thunks) slipped into the conv
            # loop. Image DMAs dispatch here (ring credit is free by then);
            # their signs follow once the transfers have landed.
            ximg = {}

            def img_dma(n, half):
                ximg[(n, half)] = x_img_dma(nc.sync, n, half)[2]

            def img_sign(n, half):
                sign_img(n, half, ximg[(n, half)])

            hooks = {
                0: [lambda: img_dma(1, 0)],
                1: [lambda: img_dma(1, 1)],
                2: [lambda: img_dma(2, 0)],
                3: [lambda: w_unit(1, w1[0], 0), lambda: img_dma(2, 1)],
                4: [lambda: img_dma(3, 0)],
                5: [lambda: w_unit(1, w1[1], 1)],
                6: [lambda: img_dma(3, 1), lambda: img_sign(1, 0)],
                7: [lambda: img_sign(1, 1)],
                8: [lambda: act_memset(2)],
                10: [lambda: img_sign(2, 0)],
                11: [lambda: img_sign(2, 1)],
                12: [lambda: act_memset(3)],
                14: [lambda: img_sign(3, 0)],
                15: [lambda: img_sign(3, 1)],
            }

            # ---- main conv loop: image-major, ot inner ----
            def conv_strip(n, ot, s, gidx, r0=0, nr=RS):
                pm = pmm_pool.tile([128, nr * W], F32)
                for kk in range(K * K):
                    kh, kw = divmod(kk, K)
                    base = (RS * s + r0 + kh) * PW + kw
                    lhsT2 = wbuf[:, :, kk * O + ot * 128 : kk * O + ot * 128 + 128]
                    src = acte if (n == 0 and s == 0) else act[:, 2 * n : 2 * n + 2]
                    # 2-level moving AP: nr rows x 56 valid cols
                    rhs4 = src[:, :, base : base + nr * PW].rearrange(
                        "p a (r w) -> p a r w", w=PW
                    )[:, :, :, :W]
                    nc.tensor.matmul(
                        pm[:],
                        lhsT2,
                        rhs4,
                        start=(kk == 0),
                        stop=(kk == K * K - 1),
                        perf_mode=mybir.MatmulPerfMode.DoubleRow,
                    )
                # evacuate, scaled by per-channel alpha
                osb = osb_pool.tile([128, nr * W], F32)
                nc.vector.tensor_scalar_mul(osb[:], pm[:], alpha_sb[:, ot : ot + 1])
                # early outs via GpSimd SWDGE (sync HWDGE is busy with
                # inputs); late outs via the then-idle sync ring
                eng = nc.gpsimd if gidx < OUT_SPLIT else nc.sync
                eng.dma_start(
                    out=out[
                        n,
                        ot * 128 : (ot + 1) * 128,
                        RS * s + r0 : RS * s + r0 + nr,
                        :,
                    ].rearrange("o h w -> o (h w)"),
                    in_=osb[:],
                )

            gidx = 0
            for n in range(NB):
                for ot in range(2):
                    for s in range(NSTRIP):
                        if gidx == 2 * NB * NSTRIP - 1:
                            # split the final strip: the first half's
                            # evac+DMA overlaps the second half's matmuls
                            conv_strip(n, ot, s, gidx, 0, RS // 2)
                            conv_strip(n, ot, s, gidx, RS // 2, RS // 2)
                        else:
                            conv_strip(n, ot, s, gidx)
                        for h in hooks.get(gidx, ()):
                            h()
                        gidx += 1
    nc.finalize()
    return nc


_NC_CACHE: dict[bool, bass.Bass] = {}


def get_nc(paired: bool = True) -> bass.Bass:
    if paired not in _NC_CACHE:
        _NC_CACHE[paired] = build_nc()
    return _NC_CACHE[paired]


def kernel(x: np.ndarray, M: np.ndarray, Alpha: np.ndarray) -> np.ndarray:
    """Full (unsharded) inputs in, full output out. Runs on 8 NeuronCores."""
    assert x.shape == (N_CORES * NB, C, H, W), x.shape
    nc = get_nc()
    x = np.ascontiguousarray(x, dtype=np.float32)
    M = np.ascontiguousarray(M, dtype=np.float32)
    a = np.ascontiguousarray(Alpha, dtype=np.float32).reshape(O)
    in_maps = [
        {"x": x[i * NB : (i + 1) * NB], "m": M, "alpha": a} for i in range(N_CORES)
    ]
    last_err = None
    for attempt in range(3):
        try:
            res = run_bass_kernel_spmd(nc, in_maps, list(range(N_CORES)))
            break
        except Exception as e:  # transient NRT/axon faults recover on retry
            last_err = e
            time.sleep(10 * (attempt + 1))
    else:
        raise last_err
    return np.concatenate([res.results[i]["out"] for i in range(N_CORES)], axis=0)
